# revision 29
# baseline (speedup 1.0000x reference)
"""GRU scan kernel for Trainium2, 8-core data-parallel.

Problem: B=64, S=512, I=512, H=1024, O=2 GRU + FC + log_softmax.

Strategy (v2): shard batch 8-way (8 rows/core). Per core, a 512-step scan
where each step streams Whh (bf16, [1024, 3072]) through the PE at 4-way
column-group concurrency (tile_position), with batch-8 stationaries.

Key layout: the "staircase" SM/ST pair, chosen so SM -> ST is exactly the
DVE's 32x32-block transpose (nc.vector.transpose):
  SM[32g+b, 32m+i] = v[b, 128m+32g+i]   (batch-major, for elementwise)
  ST[32g+i, 32m+b] = v[b, 128m+32g+i]   (feature-major; ST[:, 32k:32k+32]
                                          is the matmul stationary for
                                          contraction k-tile k)
Weights are column-permuted on the host so gate matmuls write SM directly.

Per step: r matmuls -> sigmoid -> (DVE transpose, mul with hT) -> z
matmuls -> hc matmuls (stationary r*h in ST) -> tanh -> blend in SM bf16
-> one DVE transpose of h. The sigmoid/tanh/blend chains are split in 3
free-dim parts so downstream matmuls start as soon as their k-tiles are
ready. The x @ Wx precompute (bf16, N=512 matmuls) is interleaved into
the post-candidate bubble, 2 thunks/step, writing xpart chunks to DRAM
32 steps ahead of the scan; this both hides the precompute and keeps the
PE HAM clock-gate warm.
"""

import os
import sys
from contextlib import ExitStack

for _p in ("/opt/trn_rl_repo",):
    if os.path.isdir(_p) and _p not in sys.path:
        sys.path.insert(0, _p)

import numpy as np
import ml_dtypes

import concourse.bass as bass
import concourse.mybir as mybir
import concourse.tile as tile
from concourse import bacc
from concourse.bass import ds
from concourse.bass_utils import run_bass_kernel_spmd

B, S, I, H, O = 64, 512, 512, 1024, 2
NCORES = 8
BL = B // NCORES          # 8 batch rows per core
G3 = 3 * H                # 3072 gate features, gate order [r | z | hc]
KT = H // 128             # 8 k-tiles over hidden dim
KTI = I // 128            # 4 k-tiles over input dim
F32, BF16 = mybir.dt.float32, mybir.dt.bfloat16
AFT = mybir.ActivationFunctionType
PAD_CHUNKS = 2            # precompute runs 2 chunks (32 steps) ahead
PARTS = [(0, 128), (128, 256)]  # free-dim pipeline splits


def _pcol():
    """SM column permutation: position g*256+32m+i holds gate feat 128m+32g+i."""
    p = np.empty(H, np.int64)
    for g in range(4):
        for m in range(8):
            p[g * 256 + 32 * m + np.arange(32)] = 128 * m + 32 * g + np.arange(32)
    return p


def build(n_bodies=S // 16, num_devices=NCORES):
    """Build the Bass program. n_bodies 16-step bodies (32 for the real run)."""
    nsteps = 16 * n_bodies
    n_rows = BL * nsteps
    pad_rows = 128 * PAD_CHUNKS

    nc = bacc.Bacc("TRN2", target_bir_lowering=False, debug=False,
                   num_devices=num_devices)

    xt_d = nc.dram_tensor("xt", [I, n_rows + pad_rows], BF16, kind="ExternalInput")
    xtf_d = nc.dram_tensor("xtf", [I, n_rows + pad_rows], F32, kind="ExternalInput")
    wxf_d = nc.dram_tensor("wxf", [128, KTI * G3], F32, kind="ExternalInput")
    whh_d = nc.dram_tensor("whh", [128, KT * G3], BF16, kind="ExternalInput")
    wx_d = nc.dram_tensor("wx", [128, KTI * G3], BF16, kind="ExternalInput")
    bias_d = nc.dram_tensor("bias", [1, G3], BF16, kind="ExternalInput")
    h0sm_d = nc.dram_tensor("h0sm", [128, 256], BF16, kind="ExternalInput")
    h0st_d = nc.dram_tensor("h0st", [128, 256], BF16, kind="ExternalInput")
    id8_d = nc.dram_tensor("id8", [8, 32], BF16, kind="ExternalInput")
    ones1_d = nc.dram_tensor("ones1", [1, 128], BF16, kind="ExternalInput")
    wfc_d = nc.dram_tensor("wfc", [128, KT * O], BF16, kind="ExternalInput")
    bfc_d = nc.dram_tensor("bfc", [1, O], BF16, kind="ExternalInput")
    out_d = nc.dram_tensor("out", [BL, O], F32, kind="ExternalOutput")

    xpart_d = nc.dram_tensor("xpart", [n_rows + pad_rows, G3], BF16)

    with tile.TileContext(nc) as tc, ExitStack() as ctx:
        # ---------------- constants resident in SBUF ----------------
        pconst = ctx.enter_context(tc.tile_pool(name="pconst", bufs=1))
        whh = pconst.tile([128, KT * G3], BF16)
        for k in range(KT):
            nc.sync.dma_start(out=whh[:, G3 * k : G3 * (k + 1)],
                              in_=whh_d[:, G3 * k : G3 * (k + 1)])
        wx = pconst.tile([128, KTI * G3], BF16)
        for k in range(KTI):
            nc.sync.dma_start(out=wx[:, G3 * k : G3 * (k + 1)],
                              in_=wx_d[:, G3 * k : G3 * (k + 1)])
        wxf = pconst.tile([128, KTI * G3], F32)
        for k in range(KTI):
            nc.sync.dma_start(out=wxf[:, G3 * k : G3 * (k + 1)],
                              in_=wxf_d[:, G3 * k : G3 * (k + 1)])
        bias_sb = pconst.tile([1, G3], BF16)
        nc.sync.dma_start(out=bias_sb, in_=bias_d[:, :])
        id8 = pconst.tile([8, 32], BF16)
        nc.sync.dma_start(out=id8, in_=id8_d[:, :])
        ones1 = pconst.tile([1, 128], BF16)
        nc.sync.dma_start(out=ones1, in_=ones1_d[:, :])
        wfc_sb = pconst.tile([128, KT * O], BF16)
        nc.sync.dma_start(out=wfc_sb, in_=wfc_d[:, :])
        bfc_sb = pconst.tile([1, O], BF16)
        nc.sync.dma_start(out=bfc_sb, in_=bfc_d[:, :])

        # persistent scan state
        hA = pconst.tile([128, 256], BF16)   # h in SM space (even steps in)
        nc.sync.dma_start(out=hA, in_=h0sm_d[:, :])
        hB = pconst.tile([128, 256], BF16)
        hT = pconst.tile([128, 256], BF16)   # h in ST space (matmul stationary)
        nc.sync.dma_start(out=hT, in_=h0st_d[:, :])

        # ---------------- pools ----------------
        pxp = ctx.enter_context(tc.tile_pool(name="pxp", bufs=3))
        pxt = ctx.enter_context(tc.tile_pool(name="pxt", bufs=2))
        pchunk = ctx.enter_context(tc.tile_pool(name="pchunk", bufs=2))
        ptmp = ctx.enter_context(tc.tile_pool(name="ptmp", bufs=1))
        pps = ctx.enter_context(tc.tile_pool(name="pps", bufs=1, space="PSUM"))
        ppps = ctx.enter_context(tc.tile_pool(name="ppps", bufs=3, space="PSUM"))

        r_ps = pps.tile([128, 512], F32, tag="r_ps")
        z_ps = pps.tile([128, 512], F32, tag="z_ps")
        hc1_ps = pps.tile([128, 512], F32, tag="hc1_ps")
        hc2_ps = pps.tile([128, 512], F32, tag="hc2_ps")
        fc_ps = pps.tile([BL, O], F32, tag="fc")

        # ---------------- precompute chunk thunks ----------------
        def make_chunk_thunks(row_expr):
            """Emit thunks computing xpart rows [row_expr, row_expr+128).

            pe thunks: 1 dma + 12 matmul groups (drain ~2/step).
            act thunks: 6 psum->sbuf copies + 1 dma out (drain 1/step, u>=2).
            """
            st = {}

            def dma_xtf():
                t = pxt.tile([128, KTI, 128], F32, tag="xtf")
                for k in range(KTI):
                    nc.sync.dma_start(
                        out=t[:, k, :],
                        in_=xtf_d[128 * k : 128 * (k + 1), ds(row_expr, 128)])
                st["xtf"] = t
                xpc = pchunk.tile([128, G3], BF16, tag="xpc")
                st["xpc"] = xpc
                st["pp"] = {}

            def dma_xtb():
                t = pxt.tile([128, KTI, 128], BF16, tag="xtb")
                for k in range(KTI):
                    nc.sync.dma_start(
                        out=t[:, k, :],
                        in_=xt_d[128 * k : 128 * (k + 1), ds(row_expr, 128)])
                st["xtb"] = t

            def mk_thunks(n):
                # chunks 0-3 run fp32 (4 cyc/col: real work that keeps the
                # PE busy through the post-candidate bubble); 4-5 run bf16.
                f32 = n < 4
                xtk, wxk = ("xtf", wxf) if f32 else ("xtb", wx)
                out = []

                def mm_bias(n=n):
                    pp = ppps.tile([128, 512], F32, tag="pp")
                    st["pp"][n] = pp
                    nc.tensor.matmul(pp, ones1,
                                     bias_sb[:, 512 * n : 512 * (n + 1)],
                                     start=True, stop=False)
                out.append(mm_bias)
                for k in range(KTI):
                    def mm_k(n=n, k=k, xtk=xtk, wxk=wxk):
                        nc.tensor.matmul(
                            st["pp"][n], st[xtk][:, k, :],
                            wxk[:, G3 * k + 512 * n : G3 * k + 512 * (n + 1)],
                            start=False, stop=(k == KTI - 1))
                    out.append(mm_k)
                return out

            fth = [t for n in range(4) for t in mk_thunks(n)]
            bth = [t for n in range(4, 6) for t in mk_thunks(n)]
            pe = [dma_xtf, dma_xtb]
            for g in range(10):
                pe += [fth[2 * g], fth[2 * g + 1], bth[g]]

            # act schedule keyed by step u; paced for ppps bufs=3 rotation
            copies = {}
            for n in range(6):
                def cp(n=n):
                    nc.scalar.copy(st["xpc"][:, 512 * n : 512 * (n + 1)],
                                   st["pp"][n])
                copies[n] = cp
            act = {}
            for n, u in ((0, 5), (1, 8), (4, 8), (2, 12), (3, 15), (5, 15)):
                act.setdefault(u, []).append(copies[n])

            def dma_out():
                nc.sync.dma_start(out=xpart_d[ds(row_expr, 128), :],
                                  in_=st["xpc"])
            act.setdefault(15, []).append(dma_out)
            return pe, act, copies, dma_out

        # ---------------- one scan step ----------------
        def mm_init(gt, ps, xpf):
            for g in range(4):
                nc.tensor.matmul(
                    ps[32 * g : 32 * g + 32, :256], id8,
                    xpf[:, 1024 * gt + 256 * g : 1024 * gt + 256 * (g + 1)],
                    start=True, stop=False, tile_position=(0, 32 * g),
                    skip_group_check=True)

        def mm_gate(gt, ps, statT):
            for kc in range(KT):
                for g in range(4):
                    nc.tensor.matmul(
                        ps[32 * g : 32 * g + 32, :256],
                        statT[:, 32 * kc : 32 * kc + 32],
                        whh[:, G3 * kc + 1024 * gt + 256 * g :
                            G3 * kc + 1024 * gt + 256 * (g + 1)],
                        start=False, stop=(kc == KT - 1),
                        tile_position=(0, 32 * g), skip_group_check=True)

        def dummy_mm(gate_ap):
            """Tiny matmul gated on a chain tensor — keeps the PE HAM-warm
            through the post-candidate bubble without doing real work."""
            nc.tensor.matmul(fc_ps, id8[:, :BL], gate_ap,
                             start=True, stop=True, skip_group_check=True)

        def emit_step(u, row_expr, pe_fill, act_thunks):
            hprev, hnew = (hA, hB) if u % 2 == 0 else (hB, hA)
            hc_ps = hc1_ps if u % 2 == 0 else hc2_ps

            xp = pxp.tile([8, G3], BF16, tag="xp")
            nc.sync.dma_start(out=xp, in_=xpart_d[ds(row_expr, 8), :])

            # ---- bubble fill: inits + two precompute thunks
            mm_init(0, r_ps, xp)
            mm_init(1, z_ps, xp)
            mm_init(2, hc_ps, xp)
            for _ in range(2):
                if pe_fill:
                    pe_fill.pop(0)()

            mm_gate(0, r_ps, hT)

            sr = ptmp.tile([128, 256], BF16, tag="sr")
            rt = ptmp.tile([128, 256], BF16, tag="rt")
            rh = ptmp.tile([128, 256], BF16, tag="rh")
            for a, b in PARTS:
                nc.scalar.activation(sr[:, a:b], r_ps[:, a:b], AFT.Sigmoid)
            for a, b in PARTS:
                nc.vector.transpose(rt[:, a:b], sr[:, a:b])
                nc.vector.tensor_mul(rh[:, a:b], rt[:, a:b], hT[:, a:b])

            mm_gate(1, z_ps, hT)

            zsm = ptmp.tile([128, 256], BF16, tag="zsm")
            nc.scalar.activation(zsm, z_ps[:, :256], AFT.Sigmoid)

            # v = (1-z)*h, computed off the critical path
            ww = ptmp.tile([128, 256], BF16, tag="ww")
            vv = ptmp.tile([128, 256], BF16, tag="vv")
            nc.vector.tensor_mul(ww, zsm, hprev)
            nc.vector.tensor_sub(vv, hprev, ww)

            mm_gate(2, hc_ps, rh)

            hcs = ptmp.tile([128, 256], BF16, tag="hcs")
            for a, b in PARTS:
                nc.scalar.activation(hcs[:, a:b], hc_ps[:, a:b], AFT.Tanh)
            for th in act_thunks or ():
                th()

            # h = v + z*hc, in two parts; transpose each part as it lands.
            # Dummy matmuls gated on chain tensors keep HAM at full clock.
            qq = ptmp.tile([128, 256], BF16, tag="qq")
            for a, b in PARTS:
                nc.vector.tensor_mul(qq[:, a:b], zsm[:, a:b], hcs[:, a:b])
                nc.vector.tensor_add(hnew[:, a:b], vv[:, a:b], qq[:, a:b])
                nc.vector.transpose(hT[:, a:b], hnew[:, a:b])
            for gate in (hcs[:8, 0:2], qq[:8, 0:2], hnew[:8, 0:2]):
                dummy_mm(gate)

        # ---------------- prefix: chunks 0, 1 ----------------
        # copy n emitted right after its last matmul thunk (pe-list index)
        cp_after = {8: 0, 15: 1, 16: 4, 23: 2, 30: 3, 31: 5}
        for c in range(PAD_CHUNKS):
            pe, act, copies, dma_out = make_chunk_thunks(128 * c)
            for i, th in enumerate(pe):
                th()
                if i in cp_after:
                    copies[cp_after[i]]()
            dma_out()

        # ---------------- scan ----------------
        with tc.For_i(0, n_rows, 128) as iv:
            pe_fill, act_sched, _, _ = make_chunk_thunks(iv + 128 * PAD_CHUNKS)
            for u in range(16):
                emit_step(u, iv + 8 * u, pe_fill, act_sched.get(u))
            assert not pe_fill

        # ---------------- FC head + log_softmax ----------------
        hrelu = ptmp.tile([128, 256], BF16, tag="hrelu")
        nc.scalar.activation(hrelu, hT, AFT.Relu)

        nc.tensor.matmul(fc_ps, ones1[:, :BL], bfc_sb, start=True, stop=False)
        for kc in range(KT):
            nc.tensor.matmul(fc_ps, hrelu[:, 32 * kc : 32 * kc + BL],
                             wfc_sb[:, O * kc : O * (kc + 1)],
                             start=False, stop=(kc == KT - 1))

        mx = ptmp.tile([BL, 1], F32, tag="mx")
        nc.vector.tensor_reduce(mx, fc_ps, mybir.AxisListType.X,
                                mybir.AluOpType.max)
        tt = ptmp.tile([BL, O], F32, tag="tt")
        nc.vector.tensor_scalar(tt, fc_ps, mx, None, mybir.AluOpType.subtract)
        ex = ptmp.tile([BL, O], F32, tag="ex")
        nc.scalar.activation(ex, tt, AFT.Exp)
        sm = ptmp.tile([BL, 1], F32, tag="sm")
        nc.vector.tensor_reduce(sm, ex, mybir.AxisListType.X,
                                mybir.AluOpType.add)
        lsm = ptmp.tile([BL, 1], F32, tag="lsm")
        nc.scalar.activation(lsm, sm, AFT.Ln)
        res = ptmp.tile([BL, O], F32, tag="res")
        nc.vector.tensor_scalar(res, tt, lsm, None, mybir.AluOpType.subtract)
        nc.sync.dma_start(out=out_d[:, :], in_=res)

    nc.compile()
    return nc


def prep_inputs(x, h, Wz, bz, Wr, br, Wh, bh, Wfc, bfc, nsteps=S):
    """Host-side prep: shard + relayout. Returns per-core input maps."""
    f32, bf16 = np.float32, ml_dtypes.bfloat16
    x = np.asarray(x, f32)[:, :nsteps, :]
    h0 = np.asarray(h, f32)[:, 0, :]
    pcol = _pcol()
    pad_rows = 128 * PAD_CHUNKS

    gates_h = [np.asarray(Wr, f32)[I:], np.asarray(Wz, f32)[I:],
               np.asarray(Wh, f32)[I:]]
    gates_x = [np.asarray(Wr, f32)[:I], np.asarray(Wz, f32)[:I],
               np.asarray(Wh, f32)[:I]]
    gates_b = [np.asarray(br, f32), np.asarray(bz, f32), np.asarray(bh, f32)]

    whh_img = np.zeros((128, KT * G3), bf16)
    for kc in range(KT):
        for gt in range(3):
            whh_img[:, G3 * kc + 1024 * gt : G3 * kc + 1024 * (gt + 1)] = \
                gates_h[gt][128 * kc : 128 * (kc + 1), pcol]
    wx_imgf = np.zeros((128, KTI * G3), f32)
    for k in range(KTI):
        for gt in range(3):
            wx_imgf[:, G3 * k + 1024 * gt : G3 * k + 1024 * (gt + 1)] = \
                gates_x[gt][128 * k : 128 * (k + 1), pcol]
    wx_img = wx_imgf.astype(bf16)
    bias_img = np.concatenate([g[pcol] for g in gates_b])[None, :].astype(bf16)

    id8 = np.zeros((8, 32), bf16)
    np.fill_diagonal(id8[:, :8], 1)
    ones1 = np.ones((1, 128), bf16)
    wfc_img = np.asarray(Wfc, f32).reshape(KT, 128, O).transpose(1, 0, 2) \
        .reshape(128, KT * O).astype(bf16)
    bfc_img = np.asarray(bfc, f32)[None, :].astype(bf16)

    in_maps = []
    for c in range(NCORES):
        xc = x[c * BL : (c + 1) * BL]                      # [8, S', I]
        xtf = np.zeros((I, BL * nsteps + pad_rows), f32)
        xtf[:, : BL * nsteps] = xc.transpose(2, 1, 0).reshape(I, nsteps * BL)
        xt = xtf.astype(bf16)
        h0c = h0[c * BL : (c + 1) * BL]                    # [8, H]
        hv = h0c.reshape(BL, 8, 4, 32)                     # [b, m, g, i]
        h0sm = np.zeros((128, 256), bf16)
        h0st = np.zeros((128, 256), bf16)
        for g in range(4):
            h0sm[32 * g : 32 * g + BL, :] = hv[:, :, g, :].reshape(BL, 256)
            zt = np.zeros((32, 8, 32), f32)
            zt[:, :, :BL] = hv[:, :, g, :].transpose(2, 1, 0)
            h0st[32 * g : 32 * g + 32, :] = zt.reshape(32, 256)
        in_maps.append({
            "xt": xt, "xtf": xtf, "h0sm": h0sm, "h0st": h0st,
            "whh": whh_img, "wx": wx_img, "wxf": wx_imgf, "bias": bias_img,
            "id8": id8, "ones1": ones1,
            "wfc": wfc_img, "bfc": bfc_img,
        })
    return in_maps


_BUILT = {}
_LAST_RESULTS = None


def kernel(**inputs):
    global _LAST_RESULTS
    key = "full"
    if key not in _BUILT:
        _BUILT[key] = build()
    nc = _BUILT[key]
    in_maps = prep_inputs(**inputs)
    trace = bool(int(os.environ.get("BASS_TRACE", "0") or "0"))
    res = run_bass_kernel_spmd(nc, in_maps, list(range(NCORES)), trace=trace)
    _LAST_RESULTS = res
    outs = [res.results[c]["out"] for c in range(NCORES)]
    return np.concatenate(outs, axis=0).astype(np.float32)


if __name__ == "__main__":
    np.random.seed(0)
    print("building...")
    nc = build(2, num_devices=1)
    print("build ok:", nc)


# revision 34
# speedup vs baseline: 1.1083x; 1.1083x over previous
"""GRU scan kernel for Trainium2, 8-core data-parallel.

Problem: B=64, S=512, I=512, H=1024, O=2 GRU + FC + log_softmax.

Strategy (v2): shard batch 8-way (8 rows/core). Per core, a 512-step scan
where each step streams Whh (bf16, [1024, 3072]) through the PE at 4-way
column-group concurrency (tile_position), with batch-8 stationaries.

Key layout: the "staircase" SM/ST pair, chosen so SM -> ST is exactly the
DVE's 32x32-block transpose (nc.vector.transpose):
  SM[32g+b, 32m+i] = v[b, 128m+32g+i]   (batch-major, for elementwise)
  ST[32g+i, 32m+b] = v[b, 128m+32g+i]   (feature-major; ST[:, 32k:32k+32]
                                          is the matmul stationary for
                                          contraction k-tile k)
Weights are column-permuted on the host so gate matmuls write SM directly.

Per step: r matmuls -> sigmoid -> (DVE transpose, mul with hT) -> z
matmuls -> hc matmuls (stationary r*h in ST) -> tanh -> blend in SM bf16
-> one DVE transpose of h. The sigmoid/tanh/blend chains are split in 3
free-dim parts so downstream matmuls start as soon as their k-tiles are
ready. The x @ Wx precompute (bf16, N=512 matmuls) is interleaved into
the post-candidate bubble, 2 thunks/step, writing xpart chunks to DRAM
32 steps ahead of the scan; this both hides the precompute and keeps the
PE HAM clock-gate warm.
"""

import os
import sys
from contextlib import ExitStack

for _p in ("/opt/trn_rl_repo",):
    if os.path.isdir(_p) and _p not in sys.path:
        sys.path.insert(0, _p)

import numpy as np
import ml_dtypes

import concourse.bass as bass
import concourse.mybir as mybir
import concourse.tile as tile
from concourse import bacc
from concourse.bass import ds
from concourse.bass_utils import run_bass_kernel_spmd

B, S, I, H, O = 64, 512, 512, 1024, 2
NCORES = 8
BL = B // NCORES          # 8 batch rows per core
G3 = 3 * H                # 3072 gate features, gate order [r | z | hc]
KT = H // 128             # 8 k-tiles over hidden dim
KTI = I // 128            # 4 k-tiles over input dim
F32, BF16 = mybir.dt.float32, mybir.dt.bfloat16
AFT = mybir.ActivationFunctionType
PAD_CHUNKS = 2            # precompute runs 2 chunks (32 steps) ahead
PARTS = [(0, 128), (128, 256)]  # free-dim pipeline splits


def _pcol():
    """SM column permutation: position g*256+32m+i holds gate feat 128m+32g+i."""
    p = np.empty(H, np.int64)
    for g in range(4):
        for m in range(8):
            p[g * 256 + 32 * m + np.arange(32)] = 128 * m + 32 * g + np.arange(32)
    return p


def build(n_bodies=S // 16, num_devices=NCORES):
    """Build the Bass program. n_bodies 16-step bodies (32 for the real run)."""
    nsteps = 16 * n_bodies
    n_rows = BL * nsteps
    pad_rows = 128 * PAD_CHUNKS

    nc = bacc.Bacc("TRN2", target_bir_lowering=False, debug=False,
                   num_devices=num_devices)

    xt_d = nc.dram_tensor("xt", [I, n_rows + pad_rows], BF16, kind="ExternalInput")
    xtf_d = nc.dram_tensor("xtf", [I, n_rows + pad_rows], F32, kind="ExternalInput")
    wxf_d = nc.dram_tensor("wxf", [128, KTI * G3], F32, kind="ExternalInput")
    whh_d = nc.dram_tensor("whh", [128, KT * G3], BF16, kind="ExternalInput")
    wx_d = nc.dram_tensor("wx", [128, KTI * G3], BF16, kind="ExternalInput")
    bias_d = nc.dram_tensor("bias", [1, G3], BF16, kind="ExternalInput")
    h0sm_d = nc.dram_tensor("h0sm", [128, 256], BF16, kind="ExternalInput")
    h0st_d = nc.dram_tensor("h0st", [128, 256], BF16, kind="ExternalInput")
    id8_d = nc.dram_tensor("id8", [8, 32], BF16, kind="ExternalInput")
    ones1_d = nc.dram_tensor("ones1", [1, 128], BF16, kind="ExternalInput")
    wfc_d = nc.dram_tensor("wfc", [128, KT * O], BF16, kind="ExternalInput")
    bfc_d = nc.dram_tensor("bfc", [1, O], BF16, kind="ExternalInput")
    out_d = nc.dram_tensor("out", [BL, O], F32, kind="ExternalOutput")

    xpart_d = nc.dram_tensor("xpart", [n_rows + pad_rows, G3], BF16)

    with tile.TileContext(nc) as tc, ExitStack() as ctx:
        # ---------------- constants resident in SBUF ----------------
        pconst = ctx.enter_context(tc.tile_pool(name="pconst", bufs=1))
        whh = pconst.tile([128, KT * G3], BF16)
        for k in range(KT):
            nc.sync.dma_start(out=whh[:, G3 * k : G3 * (k + 1)],
                              in_=whh_d[:, G3 * k : G3 * (k + 1)])
        wx = pconst.tile([128, KTI * G3], BF16)
        for k in range(KTI):
            nc.sync.dma_start(out=wx[:, G3 * k : G3 * (k + 1)],
                              in_=wx_d[:, G3 * k : G3 * (k + 1)])
        wxf = pconst.tile([128, KTI * G3], F32)
        for k in range(KTI):
            nc.sync.dma_start(out=wxf[:, G3 * k : G3 * (k + 1)],
                              in_=wxf_d[:, G3 * k : G3 * (k + 1)])
        bias_sb = pconst.tile([1, G3], BF16)
        nc.sync.dma_start(out=bias_sb, in_=bias_d[:, :])
        id8 = pconst.tile([8, 32], BF16)
        nc.sync.dma_start(out=id8, in_=id8_d[:, :])
        ones1 = pconst.tile([1, 128], BF16)
        nc.sync.dma_start(out=ones1, in_=ones1_d[:, :])
        wfc_sb = pconst.tile([128, KT * O], BF16)
        nc.sync.dma_start(out=wfc_sb, in_=wfc_d[:, :])
        bfc_sb = pconst.tile([1, O], BF16)
        nc.sync.dma_start(out=bfc_sb, in_=bfc_d[:, :])

        # persistent scan state
        hA = pconst.tile([128, 256], BF16)   # h in SM space (even steps in)
        nc.sync.dma_start(out=hA, in_=h0sm_d[:, :])
        hB = pconst.tile([128, 256], BF16)
        hT = pconst.tile([128, 256], BF16)   # h in ST space (matmul stationary)
        nc.sync.dma_start(out=hT, in_=h0st_d[:, :])

        # ---------------- pools ----------------
        pxp = ctx.enter_context(tc.tile_pool(name="pxp", bufs=3))
        pxt = ctx.enter_context(tc.tile_pool(name="pxt", bufs=2))
        pchunk = ctx.enter_context(tc.tile_pool(name="pchunk", bufs=2))
        ptmp = ctx.enter_context(tc.tile_pool(name="ptmp", bufs=1))
        pps = ctx.enter_context(tc.tile_pool(name="pps", bufs=1, space="PSUM"))
        ppps = ctx.enter_context(tc.tile_pool(name="ppps", bufs=3, space="PSUM"))

        r_ps = pps.tile([128, 512], F32, tag="r_ps")
        z_ps = pps.tile([128, 512], F32, tag="z_ps")
        hc1_ps = pps.tile([128, 512], F32, tag="hc1_ps")
        hc2_ps = pps.tile([128, 512], F32, tag="hc2_ps")
        fc_ps = pps.tile([BL, O], F32, tag="fc")

        # ---------------- precompute chunk thunks ----------------
        def make_chunk_thunks(row_expr):
            """Emit thunks computing xpart rows [row_expr, row_expr+128).

            pe thunks: 1 dma + 12 matmul groups (drain ~2/step).
            act thunks: 6 psum->sbuf copies + 1 dma out (drain 1/step, u>=2).
            """
            st = {}

            def dma_xtf():
                t = pxt.tile([128, KTI, 128], F32, tag="xtf")
                for k in range(KTI):
                    nc.sync.dma_start(
                        out=t[:, k, :],
                        in_=xtf_d[128 * k : 128 * (k + 1), ds(row_expr, 128)])
                st["xtf"] = t
                xpc = pchunk.tile([128, G3], BF16, tag="xpc")
                st["xpc"] = xpc
                st["pp"] = {}

            def dma_xtb():
                t = pxt.tile([128, KTI, 128], BF16, tag="xtb")
                for k in range(KTI):
                    nc.sync.dma_start(
                        out=t[:, k, :],
                        in_=xt_d[128 * k : 128 * (k + 1), ds(row_expr, 128)])
                st["xtb"] = t

            def mk_thunks(n):
                # chunks 0-1 run fp32 (4 cyc/col: real work that keeps the
                # PE busy through the post-candidate bubble); 2-5 run bf16.
                f32 = n < 2
                xtk, wxk = ("xtf", wxf) if f32 else ("xtb", wx)
                out = []

                def mm_bias(n=n):
                    pp = ppps.tile([128, 512], F32, tag="pp")
                    st["pp"][n] = pp
                    nc.tensor.matmul(pp, ones1,
                                     bias_sb[:, 512 * n : 512 * (n + 1)],
                                     start=True, stop=False)
                out.append(mm_bias)
                for k in range(KTI):
                    def mm_k(n=n, k=k, xtk=xtk, wxk=wxk):
                        nc.tensor.matmul(
                            st["pp"][n], st[xtk][:, k, :],
                            wxk[:, G3 * k + 512 * n : G3 * k + 512 * (n + 1)],
                            start=False, stop=(k == KTI - 1))
                    out.append(mm_k)
                return out

            fth = [t for n in range(2) for t in mk_thunks(n)]
            bth = [t for n in range(2, 6) for t in mk_thunks(n)]
            pe = [dma_xtf, dma_xtb]
            for g in range(10):
                pe += [fth[g], bth[2 * g], bth[2 * g + 1]]

            # act schedule keyed by step u; paced for ppps bufs=3 rotation
            copies = {}
            for n in range(6):
                def cp(n=n):
                    nc.scalar.copy(st["xpc"][:, 512 * n : 512 * (n + 1)],
                                   st["pp"][n])
                copies[n] = cp
            act = {}
            for n, u in ((0, 7), (1, 15), (2, 5), (3, 9), (4, 13), (5, 15)):
                act.setdefault(u, []).append(copies[n])

            def dma_out():
                nc.sync.dma_start(out=xpart_d[ds(row_expr, 128), :],
                                  in_=st["xpc"])
            act.setdefault(15, []).append(dma_out)
            return pe, act, copies, dma_out

        # ---------------- one scan step ----------------
        def mm_init(gt, ps, xpf):
            for g in range(4):
                nc.tensor.matmul(
                    ps[32 * g : 32 * g + 32, :256], id8,
                    xpf[:, 1024 * gt + 256 * g : 1024 * gt + 256 * (g + 1)],
                    start=True, stop=False, tile_position=(0, 32 * g),
                    skip_group_check=True)

        def mm_gate(gt, ps, statT):
            for kc in range(KT):
                for g in range(4):
                    nc.tensor.matmul(
                        ps[32 * g : 32 * g + 32, :256],
                        statT[:, 32 * kc : 32 * kc + 32],
                        whh[:, G3 * kc + 1024 * gt + 256 * g :
                            G3 * kc + 1024 * gt + 256 * (g + 1)],
                        start=False, stop=(kc == KT - 1),
                        tile_position=(0, 32 * g), skip_group_check=True)

        def dummy_mm(gate_ap):
            """Tiny matmul gated on a chain tensor — keeps the PE HAM-warm
            through the post-candidate bubble without doing real work."""
            nc.tensor.matmul(fc_ps, id8[:, :BL], gate_ap,
                             start=True, stop=True, skip_group_check=True)

        def emit_step(u, row_expr, pe_fill, act_thunks):
            hprev, hnew = (hA, hB) if u % 2 == 0 else (hB, hA)
            hc_ps = hc1_ps if u % 2 == 0 else hc2_ps

            xp = pxp.tile([8, G3], BF16, tag="xp")
            nc.sync.dma_start(out=xp, in_=xpart_d[ds(row_expr, 8), :])

            # ---- bubble fill: inits + one precompute thunk (the chain-gated
            # dummies emitted later keep the PE warm through the chain tail)
            mm_init(0, r_ps, xp)
            mm_init(1, z_ps, xp)
            mm_init(2, hc_ps, xp)
            if pe_fill:
                pe_fill.pop(0)()

            mm_gate(0, r_ps, hT)

            sr = ptmp.tile([128, 256], BF16, tag="sr")
            rt = ptmp.tile([128, 256], BF16, tag="rt")
            rh = ptmp.tile([128, 256], BF16, tag="rh")
            for a, b in PARTS:
                nc.scalar.activation(sr[:, a:b], r_ps[:, a:b], AFT.Sigmoid)
            for a, b in PARTS:
                nc.vector.transpose(rt[:, a:b], sr[:, a:b])
                nc.vector.tensor_mul(rh[:, a:b], rt[:, a:b], hT[:, a:b])

            mm_gate(1, z_ps, hT)
            if pe_fill:
                pe_fill.pop(0)()

            zsm = ptmp.tile([128, 256], BF16, tag="zsm")
            nc.scalar.activation(zsm, z_ps[:, :256], AFT.Sigmoid)

            # v = (1-z)*h, computed off the critical path
            ww = ptmp.tile([128, 256], BF16, tag="ww")
            vv = ptmp.tile([128, 256], BF16, tag="vv")
            nc.vector.tensor_mul(ww, zsm, hprev)
            nc.vector.tensor_sub(vv, hprev, ww)

            mm_gate(2, hc_ps, rh)

            hcs = ptmp.tile([128, 256], BF16, tag="hcs")
            for a, b in PARTS:
                nc.scalar.activation(hcs[:, a:b], hc_ps[:, a:b], AFT.Tanh)
            for th in act_thunks or ():
                th()

            # h = v + z*hc, in two parts; transpose each part as it lands.
            # Dummy matmuls gated on chain tensors keep HAM at full clock.
            qq = ptmp.tile([128, 256], BF16, tag="qq")
            for a, b in PARTS:
                nc.vector.tensor_mul(qq[:, a:b], zsm[:, a:b], hcs[:, a:b])
                nc.vector.tensor_add(hnew[:, a:b], vv[:, a:b], qq[:, a:b])
                nc.vector.transpose(hT[:, a:b], hnew[:, a:b])
            for gate in (hcs[:8, 0:2], qq[:8, 0:2], hnew[:8, 0:2]):
                dummy_mm(gate)

        # ---------------- prefix: chunks 0, 1 ----------------
        # copy n emitted right after its last matmul thunk (pe-list index)
        cp_after = {14: 0, 29: 1, 9: 2, 16: 3, 24: 4, 31: 5}
        for c in range(PAD_CHUNKS):
            pe, act, copies, dma_out = make_chunk_thunks(128 * c)
            for i, th in enumerate(pe):
                th()
                if i in cp_after:
                    copies[cp_after[i]]()
            dma_out()

        # ---------------- scan ----------------
        with tc.For_i(0, n_rows, 128) as iv:
            pe_fill, act_sched, _, _ = make_chunk_thunks(iv + 128 * PAD_CHUNKS)
            for u in range(16):
                emit_step(u, iv + 8 * u, pe_fill, act_sched.get(u))
            assert not pe_fill

        # ---------------- FC head + log_softmax ----------------
        hrelu = ptmp.tile([128, 256], BF16, tag="hrelu")
        nc.scalar.activation(hrelu, hT, AFT.Relu)

        nc.tensor.matmul(fc_ps, ones1[:, :BL], bfc_sb, start=True, stop=False)
        for kc in range(KT):
            nc.tensor.matmul(fc_ps, hrelu[:, 32 * kc : 32 * kc + BL],
                             wfc_sb[:, O * kc : O * (kc + 1)],
                             start=False, stop=(kc == KT - 1))

        mx = ptmp.tile([BL, 1], F32, tag="mx")
        nc.vector.tensor_reduce(mx, fc_ps, mybir.AxisListType.X,
                                mybir.AluOpType.max)
        tt = ptmp.tile([BL, O], F32, tag="tt")
        nc.vector.tensor_scalar(tt, fc_ps, mx, None, mybir.AluOpType.subtract)
        ex = ptmp.tile([BL, O], F32, tag="ex")
        nc.scalar.activation(ex, tt, AFT.Exp)
        sm = ptmp.tile([BL, 1], F32, tag="sm")
        nc.vector.tensor_reduce(sm, ex, mybir.AxisListType.X,
                                mybir.AluOpType.add)
        lsm = ptmp.tile([BL, 1], F32, tag="lsm")
        nc.scalar.activation(lsm, sm, AFT.Ln)
        res = ptmp.tile([BL, O], F32, tag="res")
        nc.vector.tensor_scalar(res, tt, lsm, None, mybir.AluOpType.subtract)
        nc.sync.dma_start(out=out_d[:, :], in_=res)

    nc.compile()
    return nc


def prep_inputs(x, h, Wz, bz, Wr, br, Wh, bh, Wfc, bfc, nsteps=S):
    """Host-side prep: shard + relayout. Returns per-core input maps."""
    f32, bf16 = np.float32, ml_dtypes.bfloat16
    x = np.asarray(x, f32)[:, :nsteps, :]
    h0 = np.asarray(h, f32)[:, 0, :]
    pcol = _pcol()
    pad_rows = 128 * PAD_CHUNKS

    gates_h = [np.asarray(Wr, f32)[I:], np.asarray(Wz, f32)[I:],
               np.asarray(Wh, f32)[I:]]
    gates_x = [np.asarray(Wr, f32)[:I], np.asarray(Wz, f32)[:I],
               np.asarray(Wh, f32)[:I]]
    gates_b = [np.asarray(br, f32), np.asarray(bz, f32), np.asarray(bh, f32)]

    whh_img = np.zeros((128, KT * G3), bf16)
    for kc in range(KT):
        for gt in range(3):
            whh_img[:, G3 * kc + 1024 * gt : G3 * kc + 1024 * (gt + 1)] = \
                gates_h[gt][128 * kc : 128 * (kc + 1), pcol]
    wx_imgf = np.zeros((128, KTI * G3), f32)
    for k in range(KTI):
        for gt in range(3):
            wx_imgf[:, G3 * k + 1024 * gt : G3 * k + 1024 * (gt + 1)] = \
                gates_x[gt][128 * k : 128 * (k + 1), pcol]
    wx_img = wx_imgf.astype(bf16)
    bias_img = np.concatenate([g[pcol] for g in gates_b])[None, :].astype(bf16)

    id8 = np.zeros((8, 32), bf16)
    np.fill_diagonal(id8[:, :8], 1)
    ones1 = np.ones((1, 128), bf16)
    wfc_img = np.asarray(Wfc, f32).reshape(KT, 128, O).transpose(1, 0, 2) \
        .reshape(128, KT * O).astype(bf16)
    bfc_img = np.asarray(bfc, f32)[None, :].astype(bf16)

    in_maps = []
    for c in range(NCORES):
        xc = x[c * BL : (c + 1) * BL]                      # [8, S', I]
        xtf = np.zeros((I, BL * nsteps + pad_rows), f32)
        xtf[:, : BL * nsteps] = xc.transpose(2, 1, 0).reshape(I, nsteps * BL)
        xt = xtf.astype(bf16)
        h0c = h0[c * BL : (c + 1) * BL]                    # [8, H]
        hv = h0c.reshape(BL, 8, 4, 32)                     # [b, m, g, i]
        h0sm = np.zeros((128, 256), bf16)
        h0st = np.zeros((128, 256), bf16)
        for g in range(4):
            h0sm[32 * g : 32 * g + BL, :] = hv[:, :, g, :].reshape(BL, 256)
            zt = np.zeros((32, 8, 32), f32)
            zt[:, :, :BL] = hv[:, :, g, :].transpose(2, 1, 0)
            h0st[32 * g : 32 * g + 32, :] = zt.reshape(32, 256)
        in_maps.append({
            "xt": xt, "xtf": xtf, "h0sm": h0sm, "h0st": h0st,
            "whh": whh_img, "wx": wx_img, "wxf": wx_imgf, "bias": bias_img,
            "id8": id8, "ones1": ones1,
            "wfc": wfc_img, "bfc": bfc_img,
        })
    return in_maps


_BUILT = {}
_LAST_RESULTS = None


def kernel(**inputs):
    global _LAST_RESULTS
    key = "full"
    if key not in _BUILT:
        _BUILT[key] = build()
    nc = _BUILT[key]
    in_maps = prep_inputs(**inputs)
    trace = bool(int(os.environ.get("BASS_TRACE", "0") or "0"))
    res = run_bass_kernel_spmd(nc, in_maps, list(range(NCORES)), trace=trace)
    _LAST_RESULTS = res
    outs = [res.results[c]["out"] for c in range(NCORES)]
    return np.concatenate(outs, axis=0).astype(np.float32)


if __name__ == "__main__":
    np.random.seed(0)
    print("building...")
    nc = build(2, num_devices=1)
    print("build ok:", nc)


# revision 36
# speedup vs baseline: 1.1109x; 1.0024x over previous
"""GRU scan kernel for Trainium2, 8-core data-parallel.

Problem: B=64, S=512, I=512, H=1024, O=2 GRU + FC + log_softmax.

Strategy (v2): shard batch 8-way (8 rows/core). Per core, a 512-step scan
where each step streams Whh (bf16, [1024, 3072]) through the PE at 4-way
column-group concurrency (tile_position), with batch-8 stationaries.

Key layout: the "staircase" SM/ST pair, chosen so SM -> ST is exactly the
DVE's 32x32-block transpose (nc.vector.transpose):
  SM[32g+b, 32m+i] = v[b, 128m+32g+i]   (batch-major, for elementwise)
  ST[32g+i, 32m+b] = v[b, 128m+32g+i]   (feature-major; ST[:, 32k:32k+32]
                                          is the matmul stationary for
                                          contraction k-tile k)
Weights are column-permuted on the host so gate matmuls write SM directly.

Per step: r matmuls -> sigmoid -> (DVE transpose, mul with hT) -> z
matmuls -> hc matmuls (stationary r*h in ST) -> tanh -> blend in SM bf16
-> one DVE transpose of h. The sigmoid/tanh/blend chains are split in 3
free-dim parts so downstream matmuls start as soon as their k-tiles are
ready. The x @ Wx precompute (bf16, N=512 matmuls) is interleaved into
the post-candidate bubble, 2 thunks/step, writing xpart chunks to DRAM
32 steps ahead of the scan; this both hides the precompute and keeps the
PE HAM clock-gate warm.
"""

import os
import sys
from contextlib import ExitStack

for _p in ("/opt/trn_rl_repo",):
    if os.path.isdir(_p) and _p not in sys.path:
        sys.path.insert(0, _p)

import numpy as np
import ml_dtypes

import concourse.bass as bass
import concourse.mybir as mybir
import concourse.tile as tile
from concourse import bacc
from concourse.bass import ds
from concourse.bass_utils import run_bass_kernel_spmd

B, S, I, H, O = 64, 512, 512, 1024, 2
NCORES = 8
BL = B // NCORES          # 8 batch rows per core
G3 = 3 * H                # 3072 gate features, gate order [r | z | hc]
KT = H // 128             # 8 k-tiles over hidden dim
KTI = I // 128            # 4 k-tiles over input dim
F32, BF16 = mybir.dt.float32, mybir.dt.bfloat16
AFT = mybir.ActivationFunctionType
PAD_CHUNKS = 2            # precompute runs 2 chunks (32 steps) ahead
PARTS = [(0, 128), (128, 256)]  # free-dim pipeline splits


def _pcol():
    """SM column permutation: position g*256+32m+i holds gate feat 128m+32g+i."""
    p = np.empty(H, np.int64)
    for g in range(4):
        for m in range(8):
            p[g * 256 + 32 * m + np.arange(32)] = 128 * m + 32 * g + np.arange(32)
    return p


def build(n_bodies=S // 32, num_devices=NCORES):
    """Build the Bass program. n_bodies 32-step bodies (16 for the real run)."""
    nsteps = 32 * n_bodies
    n_rows = BL * nsteps
    pad_rows = 128 * PAD_CHUNKS

    nc = bacc.Bacc("TRN2", target_bir_lowering=False, debug=False,
                   num_devices=num_devices)

    xt_d = nc.dram_tensor("xt", [I, n_rows + pad_rows], BF16, kind="ExternalInput")
    xtf_d = nc.dram_tensor("xtf", [I, n_rows + pad_rows], F32, kind="ExternalInput")
    wxf_d = nc.dram_tensor("wxf", [128, KTI * G3], F32, kind="ExternalInput")
    whh_d = nc.dram_tensor("whh", [128, KT * G3], BF16, kind="ExternalInput")
    wx_d = nc.dram_tensor("wx", [128, KTI * G3], BF16, kind="ExternalInput")
    bias_d = nc.dram_tensor("bias", [1, G3], BF16, kind="ExternalInput")
    h0sm_d = nc.dram_tensor("h0sm", [128, 256], BF16, kind="ExternalInput")
    h0st_d = nc.dram_tensor("h0st", [128, 256], BF16, kind="ExternalInput")
    id8_d = nc.dram_tensor("id8", [8, 32], BF16, kind="ExternalInput")
    ones1_d = nc.dram_tensor("ones1", [1, 128], BF16, kind="ExternalInput")
    wfc_d = nc.dram_tensor("wfc", [128, KT * O], BF16, kind="ExternalInput")
    bfc_d = nc.dram_tensor("bfc", [1, O], BF16, kind="ExternalInput")
    out_d = nc.dram_tensor("out", [BL, O], F32, kind="ExternalOutput")

    xpart_d = nc.dram_tensor("xpart", [n_rows + pad_rows, G3], BF16)

    with tile.TileContext(nc) as tc, ExitStack() as ctx:
        # ---------------- constants resident in SBUF ----------------
        pconst = ctx.enter_context(tc.tile_pool(name="pconst", bufs=1))
        whh = pconst.tile([128, KT * G3], BF16)
        for k in range(KT):
            nc.sync.dma_start(out=whh[:, G3 * k : G3 * (k + 1)],
                              in_=whh_d[:, G3 * k : G3 * (k + 1)])
        wx = pconst.tile([128, KTI * G3], BF16)
        for k in range(KTI):
            nc.sync.dma_start(out=wx[:, G3 * k : G3 * (k + 1)],
                              in_=wx_d[:, G3 * k : G3 * (k + 1)])
        wxf = pconst.tile([128, KTI * G3], F32)
        for k in range(KTI):
            nc.sync.dma_start(out=wxf[:, G3 * k : G3 * (k + 1)],
                              in_=wxf_d[:, G3 * k : G3 * (k + 1)])
        bias_sb = pconst.tile([1, G3], BF16)
        nc.sync.dma_start(out=bias_sb, in_=bias_d[:, :])
        id8 = pconst.tile([8, 32], BF16)
        nc.sync.dma_start(out=id8, in_=id8_d[:, :])
        ones1 = pconst.tile([1, 128], BF16)
        nc.sync.dma_start(out=ones1, in_=ones1_d[:, :])
        wfc_sb = pconst.tile([128, KT * O], BF16)
        nc.sync.dma_start(out=wfc_sb, in_=wfc_d[:, :])
        bfc_sb = pconst.tile([1, O], BF16)
        nc.sync.dma_start(out=bfc_sb, in_=bfc_d[:, :])

        # persistent scan state
        hA = pconst.tile([128, 256], BF16)   # h in SM space (even steps in)
        nc.sync.dma_start(out=hA, in_=h0sm_d[:, :])
        hB = pconst.tile([128, 256], BF16)
        hT = pconst.tile([128, 256], BF16)   # h in ST space (matmul stationary)
        nc.sync.dma_start(out=hT, in_=h0st_d[:, :])

        # ---------------- pools ----------------
        pxp = ctx.enter_context(tc.tile_pool(name="pxp", bufs=3))
        pxt = ctx.enter_context(tc.tile_pool(name="pxt", bufs=2))
        pchunk = ctx.enter_context(tc.tile_pool(name="pchunk", bufs=2))
        ptmp = ctx.enter_context(tc.tile_pool(name="ptmp", bufs=1))
        pps = ctx.enter_context(tc.tile_pool(name="pps", bufs=1, space="PSUM"))
        ppps = ctx.enter_context(tc.tile_pool(name="ppps", bufs=3, space="PSUM"))

        r_ps = pps.tile([128, 512], F32, tag="r_ps")
        z_ps = pps.tile([128, 512], F32, tag="z_ps")
        hc1_ps = pps.tile([128, 512], F32, tag="hc1_ps")
        hc2_ps = pps.tile([128, 512], F32, tag="hc2_ps")
        fc_ps = pps.tile([BL, O], F32, tag="fc")

        # ---------------- precompute chunk thunks ----------------
        def make_chunk_thunks(row_expr):
            """Emit thunks computing xpart rows [row_expr, row_expr+128).

            pe thunks: 1 dma + 12 matmul groups (drain ~2/step).
            act thunks: 6 psum->sbuf copies + 1 dma out (drain 1/step, u>=2).
            """
            st = {}

            def dma_xtf():
                t = pxt.tile([128, KTI, 128], F32, tag="xtf")
                for k in range(KTI):
                    nc.sync.dma_start(
                        out=t[:, k, :],
                        in_=xtf_d[128 * k : 128 * (k + 1), ds(row_expr, 128)])
                st["xtf"] = t
                xpc = pchunk.tile([128, G3], BF16, tag="xpc")
                st["xpc"] = xpc
                st["pp"] = {}

            def dma_xtb():
                t = pxt.tile([128, KTI, 128], BF16, tag="xtb")
                for k in range(KTI):
                    nc.sync.dma_start(
                        out=t[:, k, :],
                        in_=xt_d[128 * k : 128 * (k + 1), ds(row_expr, 128)])
                st["xtb"] = t

            def mk_thunks(n):
                # chunks 0-1 run fp32 (4 cyc/col: real work that keeps the
                # PE busy through the post-candidate bubble); 2-5 run bf16.
                f32 = n < 2
                xtk, wxk = ("xtf", wxf) if f32 else ("xtb", wx)
                out = []

                def mm_bias(n=n):
                    pp = ppps.tile([128, 512], F32, tag="pp")
                    st["pp"][n] = pp
                    nc.tensor.matmul(pp, ones1,
                                     bias_sb[:, 512 * n : 512 * (n + 1)],
                                     start=True, stop=False)
                out.append(mm_bias)
                for k in range(KTI):
                    def mm_k(n=n, k=k, xtk=xtk, wxk=wxk):
                        nc.tensor.matmul(
                            st["pp"][n], st[xtk][:, k, :],
                            wxk[:, G3 * k + 512 * n : G3 * k + 512 * (n + 1)],
                            start=False, stop=(k == KTI - 1))
                    out.append(mm_k)
                return out

            fth = [t for n in range(2) for t in mk_thunks(n)]
            bth = [t for n in range(2, 6) for t in mk_thunks(n)]
            pe = [dma_xtf, dma_xtb]
            for g in range(10):
                pe += [fth[g], bth[2 * g], bth[2 * g + 1]]

            # act schedule keyed by step u; paced for ppps bufs=3 rotation
            copies = {}
            for n in range(6):
                def cp(n=n):
                    nc.scalar.copy(st["xpc"][:, 512 * n : 512 * (n + 1)],
                                   st["pp"][n])
                copies[n] = cp
            act = {}
            for n, u in ((0, 7), (1, 15), (2, 5), (3, 9), (4, 13), (5, 15)):
                act.setdefault(u, []).append(copies[n])

            def dma_out():
                nc.sync.dma_start(out=xpart_d[ds(row_expr, 128), :],
                                  in_=st["xpc"])
            act.setdefault(15, []).append(dma_out)
            return pe, act, copies, dma_out

        # ---------------- one scan step ----------------
        def mm_init(gt, ps, xpf):
            for g in range(4):
                nc.tensor.matmul(
                    ps[32 * g : 32 * g + 32, :256], id8,
                    xpf[:, 1024 * gt + 256 * g : 1024 * gt + 256 * (g + 1)],
                    start=True, stop=False, tile_position=(0, 32 * g),
                    skip_group_check=True)

        def mm_gate(gt, ps, statT):
            for kc in range(KT):
                for g in range(4):
                    nc.tensor.matmul(
                        ps[32 * g : 32 * g + 32, :256],
                        statT[:, 32 * kc : 32 * kc + 32],
                        whh[:, G3 * kc + 1024 * gt + 256 * g :
                            G3 * kc + 1024 * gt + 256 * (g + 1)],
                        start=False, stop=(kc == KT - 1),
                        tile_position=(0, 32 * g), skip_group_check=True)

        def dummy_mm(gate_ap):
            """Tiny matmul gated on a chain tensor — keeps the PE HAM-warm
            through the post-candidate bubble without doing real work."""
            nc.tensor.matmul(fc_ps, id8[:, :BL], gate_ap,
                             start=True, stop=True, skip_group_check=True)

        def emit_step(u, row_expr, pe_fill, act_thunks):
            hprev, hnew = (hA, hB) if u % 2 == 0 else (hB, hA)
            hc_ps = hc1_ps if u % 2 == 0 else hc2_ps

            xp = pxp.tile([8, G3], BF16, tag="xp")
            nc.sync.dma_start(out=xp, in_=xpart_d[ds(row_expr, 8), :])

            # ---- bubble fill: inits + one precompute thunk (the chain-gated
            # dummies emitted later keep the PE warm through the chain tail)
            mm_init(0, r_ps, xp)
            mm_init(1, z_ps, xp)
            mm_init(2, hc_ps, xp)
            if pe_fill:
                pe_fill.pop(0)()

            mm_gate(0, r_ps, hT)

            sr = ptmp.tile([128, 256], BF16, tag="sr")
            rt = ptmp.tile([128, 256], BF16, tag="rt")
            rh = ptmp.tile([128, 256], BF16, tag="rh")
            for a, b in PARTS:
                nc.scalar.activation(sr[:, a:b], r_ps[:, a:b], AFT.Sigmoid)
            for a, b in PARTS:
                nc.vector.transpose(rt[:, a:b], sr[:, a:b])
                nc.vector.tensor_mul(rh[:, a:b], rt[:, a:b], hT[:, a:b])

            mm_gate(1, z_ps, hT)
            if pe_fill:
                pe_fill.pop(0)()

            zsm = ptmp.tile([128, 256], BF16, tag="zsm")
            nc.scalar.activation(zsm, z_ps[:, :256], AFT.Sigmoid)

            # v = (1-z)*h, computed off the critical path
            ww = ptmp.tile([128, 256], BF16, tag="ww")
            vv = ptmp.tile([128, 256], BF16, tag="vv")
            nc.vector.tensor_mul(ww, zsm, hprev)
            nc.vector.tensor_sub(vv, hprev, ww)

            mm_gate(2, hc_ps, rh)

            hcs = ptmp.tile([128, 256], BF16, tag="hcs")
            for a, b in PARTS:
                nc.scalar.activation(hcs[:, a:b], hc_ps[:, a:b], AFT.Tanh)
            for th in act_thunks or ():
                th()

            # h = v + z*hc, in two parts; transpose each part as it lands.
            # Dummy matmuls gated on chain tensors keep HAM at full clock.
            qq = ptmp.tile([128, 256], BF16, tag="qq")
            for a, b in PARTS:
                nc.vector.tensor_mul(qq[:, a:b], zsm[:, a:b], hcs[:, a:b])
                nc.vector.tensor_add(hnew[:, a:b], vv[:, a:b], qq[:, a:b])
                nc.vector.transpose(hT[:, a:b], hnew[:, a:b])
            for gate in (hcs[:8, 0:2], qq[:8, 0:2], hnew[:8, 0:2]):
                dummy_mm(gate)

        # ---------------- prefix: chunks 0, 1 ----------------
        # copy n emitted right after its last matmul thunk (pe-list index)
        cp_after = {14: 0, 29: 1, 9: 2, 16: 3, 24: 4, 31: 5}
        for c in range(PAD_CHUNKS):
            pe, act, copies, dma_out = make_chunk_thunks(128 * c)
            for i, th in enumerate(pe):
                th()
                if i in cp_after:
                    copies[cp_after[i]]()
            dma_out()

        # ---------------- scan ----------------
        with tc.For_i(0, n_rows, 256) as iv:
            pe_a, act_a, _, _ = make_chunk_thunks(iv + 256)
            pe_b, act_b, _, _ = make_chunk_thunks(iv + 384)
            pe_fill = pe_a + pe_b
            act_sched = dict(act_a)
            for k, v in act_b.items():
                act_sched.setdefault(k + 16, []).extend(v)
            for u in range(32):
                emit_step(u, iv + 8 * u, pe_fill, act_sched.get(u))
            assert not pe_fill

        # ---------------- FC head + log_softmax ----------------
        hrelu = ptmp.tile([128, 256], BF16, tag="hrelu")
        nc.scalar.activation(hrelu, hT, AFT.Relu)

        nc.tensor.matmul(fc_ps, ones1[:, :BL], bfc_sb, start=True, stop=False)
        for kc in range(KT):
            nc.tensor.matmul(fc_ps, hrelu[:, 32 * kc : 32 * kc + BL],
                             wfc_sb[:, O * kc : O * (kc + 1)],
                             start=False, stop=(kc == KT - 1))

        mx = ptmp.tile([BL, 1], F32, tag="mx")
        nc.vector.tensor_reduce(mx, fc_ps, mybir.AxisListType.X,
                                mybir.AluOpType.max)
        tt = ptmp.tile([BL, O], F32, tag="tt")
        nc.vector.tensor_scalar(tt, fc_ps, mx, None, mybir.AluOpType.subtract)
        ex = ptmp.tile([BL, O], F32, tag="ex")
        nc.scalar.activation(ex, tt, AFT.Exp)
        sm = ptmp.tile([BL, 1], F32, tag="sm")
        nc.vector.tensor_reduce(sm, ex, mybir.AxisListType.X,
                                mybir.AluOpType.add)
        lsm = ptmp.tile([BL, 1], F32, tag="lsm")
        nc.scalar.activation(lsm, sm, AFT.Ln)
        res = ptmp.tile([BL, O], F32, tag="res")
        nc.vector.tensor_scalar(res, tt, lsm, None, mybir.AluOpType.subtract)
        nc.sync.dma_start(out=out_d[:, :], in_=res)

    nc.compile()
    return nc


def prep_inputs(x, h, Wz, bz, Wr, br, Wh, bh, Wfc, bfc, nsteps=S):
    """Host-side prep: shard + relayout. Returns per-core input maps."""
    f32, bf16 = np.float32, ml_dtypes.bfloat16
    x = np.asarray(x, f32)[:, :nsteps, :]
    h0 = np.asarray(h, f32)[:, 0, :]
    pcol = _pcol()
    pad_rows = 128 * PAD_CHUNKS

    gates_h = [np.asarray(Wr, f32)[I:], np.asarray(Wz, f32)[I:],
               np.asarray(Wh, f32)[I:]]
    gates_x = [np.asarray(Wr, f32)[:I], np.asarray(Wz, f32)[:I],
               np.asarray(Wh, f32)[:I]]
    gates_b = [np.asarray(br, f32), np.asarray(bz, f32), np.asarray(bh, f32)]

    whh_img = np.zeros((128, KT * G3), bf16)
    for kc in range(KT):
        for gt in range(3):
            whh_img[:, G3 * kc + 1024 * gt : G3 * kc + 1024 * (gt + 1)] = \
                gates_h[gt][128 * kc : 128 * (kc + 1), pcol]
    wx_imgf = np.zeros((128, KTI * G3), f32)
    for k in range(KTI):
        for gt in range(3):
            wx_imgf[:, G3 * k + 1024 * gt : G3 * k + 1024 * (gt + 1)] = \
                gates_x[gt][128 * k : 128 * (k + 1), pcol]
    wx_img = wx_imgf.astype(bf16)
    bias_img = np.concatenate([g[pcol] for g in gates_b])[None, :].astype(bf16)

    id8 = np.zeros((8, 32), bf16)
    np.fill_diagonal(id8[:, :8], 1)
    ones1 = np.ones((1, 128), bf16)
    wfc_img = np.asarray(Wfc, f32).reshape(KT, 128, O).transpose(1, 0, 2) \
        .reshape(128, KT * O).astype(bf16)
    bfc_img = np.asarray(bfc, f32)[None, :].astype(bf16)

    in_maps = []
    for c in range(NCORES):
        xc = x[c * BL : (c + 1) * BL]                      # [8, S', I]
        xtf = np.zeros((I, BL * nsteps + pad_rows), f32)
        xtf[:, : BL * nsteps] = xc.transpose(2, 1, 0).reshape(I, nsteps * BL)
        xt = xtf.astype(bf16)
        h0c = h0[c * BL : (c + 1) * BL]                    # [8, H]
        hv = h0c.reshape(BL, 8, 4, 32)                     # [b, m, g, i]
        h0sm = np.zeros((128, 256), bf16)
        h0st = np.zeros((128, 256), bf16)
        for g in range(4):
            h0sm[32 * g : 32 * g + BL, :] = hv[:, :, g, :].reshape(BL, 256)
            zt = np.zeros((32, 8, 32), f32)
            zt[:, :, :BL] = hv[:, :, g, :].transpose(2, 1, 0)
            h0st[32 * g : 32 * g + 32, :] = zt.reshape(32, 256)
        in_maps.append({
            "xt": xt, "xtf": xtf, "h0sm": h0sm, "h0st": h0st,
            "whh": whh_img, "wx": wx_img, "wxf": wx_imgf, "bias": bias_img,
            "id8": id8, "ones1": ones1,
            "wfc": wfc_img, "bfc": bfc_img,
        })
    return in_maps


_BUILT = {}
_LAST_RESULTS = None


def kernel(**inputs):
    global _LAST_RESULTS
    key = "full"
    if key not in _BUILT:
        _BUILT[key] = build()
    nc = _BUILT[key]
    in_maps = prep_inputs(**inputs)
    trace = bool(int(os.environ.get("BASS_TRACE", "0") or "0"))
    res = run_bass_kernel_spmd(nc, in_maps, list(range(NCORES)), trace=trace)
    _LAST_RESULTS = res
    outs = [res.results[c]["out"] for c in range(NCORES)]
    return np.concatenate(outs, axis=0).astype(np.float32)


if __name__ == "__main__":
    np.random.seed(0)
    print("building...")
    nc = build(2, num_devices=1)
    print("build ok:", nc)


# revision 37
# speedup vs baseline: 1.1774x; 1.0599x over previous
"""GRU scan kernel for Trainium2, 8-core data-parallel.

Problem: B=64, S=512, I=512, H=1024, O=2 GRU + FC + log_softmax.

Strategy (v2): shard batch 8-way (8 rows/core). Per core, a 512-step scan
where each step streams Whh (bf16, [1024, 3072]) through the PE at 4-way
column-group concurrency (tile_position), with batch-8 stationaries.

Key layout: the "staircase" SM/ST pair, chosen so SM -> ST is exactly the
DVE's 32x32-block transpose (nc.vector.transpose):
  SM[32g+b, 32m+i] = v[b, 128m+32g+i]   (batch-major, for elementwise)
  ST[32g+i, 32m+b] = v[b, 128m+32g+i]   (feature-major; ST[:, 32k:32k+32]
                                          is the matmul stationary for
                                          contraction k-tile k)
Weights are column-permuted on the host so gate matmuls write SM directly.

Per step: r matmuls -> sigmoid -> (DVE transpose, mul with hT) -> z
matmuls -> hc matmuls (stationary r*h in ST) -> tanh -> blend in SM bf16
-> one DVE transpose of h. The sigmoid/tanh/blend chains are split in 3
free-dim parts so downstream matmuls start as soon as their k-tiles are
ready. The x @ Wx precompute (bf16, N=512 matmuls) is interleaved into
the post-candidate bubble, 2 thunks/step, writing xpart chunks to DRAM
32 steps ahead of the scan; this both hides the precompute and keeps the
PE HAM clock-gate warm.
"""

import os
import sys
from contextlib import ExitStack

for _p in ("/opt/trn_rl_repo",):
    if os.path.isdir(_p) and _p not in sys.path:
        sys.path.insert(0, _p)

import numpy as np
import ml_dtypes

import concourse.bass as bass
import concourse.mybir as mybir
import concourse.tile as tile
from concourse import bacc
from concourse.bass import ds
from concourse.bass_utils import run_bass_kernel_spmd

B, S, I, H, O = 64, 512, 512, 1024, 2
NCORES = 8
BL = B // NCORES          # 8 batch rows per core
G3 = 3 * H                # 3072 gate features, gate order [r | z | hc]
KT = H // 128             # 8 k-tiles over hidden dim
KTI = I // 128            # 4 k-tiles over input dim
F32, BF16 = mybir.dt.float32, mybir.dt.bfloat16
AFT = mybir.ActivationFunctionType
PAD_CHUNKS = 2            # precompute runs 2 chunks (32 steps) ahead
PARTS = [(0, 128), (128, 256)]  # free-dim pipeline splits


def _pcol():
    """SM column permutation: position g*256+32m+i holds gate feat 128m+32g+i."""
    p = np.empty(H, np.int64)
    for g in range(4):
        for m in range(8):
            p[g * 256 + 32 * m + np.arange(32)] = 128 * m + 32 * g + np.arange(32)
    return p


def build(n_bodies=S // 32, num_devices=NCORES):
    """Build the Bass program. n_bodies 32-step bodies (16 for the real run)."""
    nsteps = 32 * n_bodies
    n_rows = BL * nsteps
    pad_rows = 128 * PAD_CHUNKS

    nc = bacc.Bacc("TRN2", target_bir_lowering=False, debug=False,
                   num_devices=num_devices)

    xt_d = nc.dram_tensor("xt", [I, n_rows + pad_rows], BF16, kind="ExternalInput")
    xtf_d = nc.dram_tensor("xtf", [I, n_rows + pad_rows], F32, kind="ExternalInput")
    wxf_d = nc.dram_tensor("wxf", [128, KTI * G3], F32, kind="ExternalInput")
    whh_d = nc.dram_tensor("whh", [128, KT * G3], BF16, kind="ExternalInput")
    wx_d = nc.dram_tensor("wx", [128, KTI * G3], BF16, kind="ExternalInput")
    bias_d = nc.dram_tensor("bias", [1, G3], BF16, kind="ExternalInput")
    h0sm_d = nc.dram_tensor("h0sm", [128, 256], BF16, kind="ExternalInput")
    h0st_d = nc.dram_tensor("h0st", [128, 256], BF16, kind="ExternalInput")
    id8_d = nc.dram_tensor("id8", [8, 32], BF16, kind="ExternalInput")
    ones1_d = nc.dram_tensor("ones1", [1, 128], BF16, kind="ExternalInput")
    wfc_d = nc.dram_tensor("wfc", [128, KT * O], BF16, kind="ExternalInput")
    bfc_d = nc.dram_tensor("bfc", [1, O], BF16, kind="ExternalInput")
    out_d = nc.dram_tensor("out", [BL, O], F32, kind="ExternalOutput")

    xpart_d = nc.dram_tensor("xpart", [n_rows + pad_rows, G3], BF16)

    with tile.TileContext(nc) as tc, ExitStack() as ctx:
        # ---------------- constants resident in SBUF ----------------
        pconst = ctx.enter_context(tc.tile_pool(name="pconst", bufs=1))
        whh = pconst.tile([128, KT * G3], BF16)
        for k in range(KT):
            nc.sync.dma_start(out=whh[:, G3 * k : G3 * (k + 1)],
                              in_=whh_d[:, G3 * k : G3 * (k + 1)])
        wx = pconst.tile([128, KTI * G3], BF16)
        for k in range(KTI):
            nc.sync.dma_start(out=wx[:, G3 * k : G3 * (k + 1)],
                              in_=wx_d[:, G3 * k : G3 * (k + 1)])
        wxf = pconst.tile([128, KTI * G3], F32)
        for k in range(KTI):
            nc.sync.dma_start(out=wxf[:, G3 * k : G3 * (k + 1)],
                              in_=wxf_d[:, G3 * k : G3 * (k + 1)])
        bias_sb = pconst.tile([1, G3], BF16)
        nc.sync.dma_start(out=bias_sb, in_=bias_d[:, :])
        id8 = pconst.tile([8, 32], BF16)
        nc.sync.dma_start(out=id8, in_=id8_d[:, :])
        ones1 = pconst.tile([1, 128], BF16)
        nc.sync.dma_start(out=ones1, in_=ones1_d[:, :])
        wfc_sb = pconst.tile([128, KT * O], BF16)
        nc.sync.dma_start(out=wfc_sb, in_=wfc_d[:, :])
        bfc_sb = pconst.tile([1, O], BF16)
        nc.sync.dma_start(out=bfc_sb, in_=bfc_d[:, :])

        # persistent scan state
        hA = pconst.tile([128, 256], BF16)   # h in SM space (even steps in)
        nc.sync.dma_start(out=hA, in_=h0sm_d[:, :])
        hB = pconst.tile([128, 256], BF16)
        hT = pconst.tile([128, 256], BF16)   # h in ST space (matmul stationary)
        nc.sync.dma_start(out=hT, in_=h0st_d[:, :])

        # ---------------- pools ----------------
        pxp = ctx.enter_context(tc.tile_pool(name="pxp", bufs=3))
        pxt = ctx.enter_context(tc.tile_pool(name="pxt", bufs=2))
        pchunk = ctx.enter_context(tc.tile_pool(name="pchunk", bufs=2))
        ptmp = ctx.enter_context(tc.tile_pool(name="ptmp", bufs=1))
        pps = ctx.enter_context(tc.tile_pool(name="pps", bufs=1, space="PSUM"))
        ppps = ctx.enter_context(tc.tile_pool(name="ppps", bufs=3, space="PSUM"))

        r_ps = pps.tile([128, 512], F32, tag="r_ps")
        z_ps = pps.tile([128, 512], F32, tag="z_ps")
        hc1_ps = pps.tile([128, 512], F32, tag="hc1_ps")
        hc2_ps = pps.tile([128, 512], F32, tag="hc2_ps")
        fc_ps = pps.tile([BL, O], F32, tag="fc")

        # ---------------- precompute chunk thunks ----------------
        def make_chunk_thunks(row_expr):
            """Emit thunks computing xpart rows [row_expr, row_expr+128).

            pe thunks: 1 dma + 12 matmul groups (drain ~2/step).
            act thunks: 6 psum->sbuf copies + 1 dma out (drain 1/step, u>=2).
            """
            st = {}

            def dma_xtf():
                t = pxt.tile([128, KTI, 128], F32, tag="xtf")
                for k in range(KTI):
                    nc.sync.dma_start(
                        out=t[:, k, :],
                        in_=xtf_d[128 * k : 128 * (k + 1), ds(row_expr, 128)])
                st["xtf"] = t
                xpc = pchunk.tile([128, G3], BF16, tag="xpc")
                st["xpc"] = xpc
                st["pp"] = {}

            def dma_xtb():
                t = pxt.tile([128, KTI, 128], BF16, tag="xtb")
                for k in range(KTI):
                    nc.sync.dma_start(
                        out=t[:, k, :],
                        in_=xt_d[128 * k : 128 * (k + 1), ds(row_expr, 128)])
                st["xtb"] = t

            def mk_thunks(n):
                # all-bf16 precompute; the chain-gated dummies handle HAM
                f32 = False
                xtk, wxk = ("xtf", wxf) if f32 else ("xtb", wx)
                out = []

                def mm_bias(n=n):
                    pp = ppps.tile([128, 512], F32, tag="pp")
                    st["pp"][n] = pp
                    nc.tensor.matmul(pp, ones1,
                                     bias_sb[:, 512 * n : 512 * (n + 1)],
                                     start=True, stop=False)
                out.append(mm_bias)
                for k in range(KTI):
                    def mm_k(n=n, k=k, xtk=xtk, wxk=wxk):
                        nc.tensor.matmul(
                            st["pp"][n], st[xtk][:, k, :],
                            wxk[:, G3 * k + 512 * n : G3 * k + 512 * (n + 1)],
                            start=False, stop=(k == KTI - 1))
                    out.append(mm_k)
                return out

            fth = [t for n in range(2) for t in mk_thunks(n)]
            bth = [t for n in range(2, 6) for t in mk_thunks(n)]
            pe = [dma_xtf, dma_xtb]
            for g in range(10):
                pe += [fth[g], bth[2 * g], bth[2 * g + 1]]

            # act schedule keyed by step u; paced for ppps bufs=3 rotation
            copies = {}
            for n in range(6):
                def cp(n=n):
                    nc.scalar.copy(st["xpc"][:, 512 * n : 512 * (n + 1)],
                                   st["pp"][n])
                copies[n] = cp
            act = {}
            for n, u in ((0, 7), (1, 15), (2, 5), (3, 9), (4, 13), (5, 15)):
                act.setdefault(u, []).append(copies[n])

            def dma_out():
                nc.sync.dma_start(out=xpart_d[ds(row_expr, 128), :],
                                  in_=st["xpc"])
            act.setdefault(15, []).append(dma_out)
            return pe, act, copies, dma_out

        # ---------------- one scan step ----------------
        def mm_init(gt, ps, xpf):
            for g in range(4):
                nc.tensor.matmul(
                    ps[32 * g : 32 * g + 32, :256], id8,
                    xpf[:, 1024 * gt + 256 * g : 1024 * gt + 256 * (g + 1)],
                    start=True, stop=False, tile_position=(0, 32 * g),
                    skip_group_check=True)

        def mm_gate(gt, ps, statT):
            for kc in range(KT):
                for g in range(4):
                    nc.tensor.matmul(
                        ps[32 * g : 32 * g + 32, :256],
                        statT[:, 32 * kc : 32 * kc + 32],
                        whh[:, G3 * kc + 1024 * gt + 256 * g :
                            G3 * kc + 1024 * gt + 256 * (g + 1)],
                        start=False, stop=(kc == KT - 1),
                        tile_position=(0, 32 * g), skip_group_check=True)

        def dummy_mm(gate_ap):
            """Tiny matmul gated on a chain tensor — keeps the PE HAM-warm
            through the post-candidate bubble without doing real work."""
            nc.tensor.matmul(fc_ps, id8[:, :BL], gate_ap,
                             start=True, stop=True, skip_group_check=True)

        def emit_step(u, row_expr, pe_fill, act_thunks):
            hprev, hnew = (hA, hB) if u % 2 == 0 else (hB, hA)
            hc_ps = hc1_ps if u % 2 == 0 else hc2_ps

            xp = pxp.tile([8, G3], BF16, tag="xp")
            nc.sync.dma_start(out=xp, in_=xpart_d[ds(row_expr, 8), :])

            # ---- bubble fill: inits + one precompute thunk (the chain-gated
            # dummies emitted later keep the PE warm through the chain tail)
            mm_init(0, r_ps, xp)
            mm_init(1, z_ps, xp)
            mm_init(2, hc_ps, xp)
            if pe_fill:
                pe_fill.pop(0)()

            mm_gate(0, r_ps, hT)

            sr = ptmp.tile([128, 256], BF16, tag="sr")
            rt = ptmp.tile([128, 256], BF16, tag="rt")
            rh = ptmp.tile([128, 256], BF16, tag="rh")
            for a, b in PARTS:
                nc.scalar.activation(sr[:, a:b], r_ps[:, a:b], AFT.Sigmoid)
            for a, b in PARTS:
                nc.vector.transpose(rt[:, a:b], sr[:, a:b])
                nc.vector.tensor_mul(rh[:, a:b], rt[:, a:b], hT[:, a:b])

            mm_gate(1, z_ps, hT)
            if pe_fill:
                pe_fill.pop(0)()

            zsm = ptmp.tile([128, 256], BF16, tag="zsm")
            nc.scalar.activation(zsm, z_ps[:, :256], AFT.Sigmoid)

            # v = (1-z)*h, computed off the critical path
            ww = ptmp.tile([128, 256], BF16, tag="ww")
            vv = ptmp.tile([128, 256], BF16, tag="vv")
            nc.vector.tensor_mul(ww, zsm, hprev)
            nc.vector.tensor_sub(vv, hprev, ww)

            mm_gate(2, hc_ps, rh)

            hcs = ptmp.tile([128, 256], BF16, tag="hcs")
            for a, b in PARTS:
                nc.scalar.activation(hcs[:, a:b], hc_ps[:, a:b], AFT.Tanh)
            for th in act_thunks or ():
                th()

            # h = v + z*hc, in two parts; transpose each part as it lands.
            # Dummy matmuls gated on chain tensors keep HAM at full clock.
            qq = ptmp.tile([128, 256], BF16, tag="qq")
            for a, b in PARTS:
                nc.vector.tensor_mul(qq[:, a:b], zsm[:, a:b], hcs[:, a:b])
                nc.vector.tensor_add(hnew[:, a:b], vv[:, a:b], qq[:, a:b])
                nc.vector.transpose(hT[:, a:b], hnew[:, a:b])
            for gate in (hcs[:8, 0:2], qq[:8, 0:2], hnew[:8, 0:2]):
                dummy_mm(gate)

        # ---------------- prefix: chunks 0, 1 ----------------
        # copy n emitted right after its last matmul thunk (pe-list index)
        cp_after = {14: 0, 29: 1, 9: 2, 16: 3, 24: 4, 31: 5}
        for c in range(PAD_CHUNKS):
            pe, act, copies, dma_out = make_chunk_thunks(128 * c)
            for i, th in enumerate(pe):
                th()
                if i in cp_after:
                    copies[cp_after[i]]()
            dma_out()

        # ---------------- scan ----------------
        with tc.For_i(0, n_rows, 256) as iv:
            pe_a, act_a, _, _ = make_chunk_thunks(iv + 256)
            pe_b, act_b, _, _ = make_chunk_thunks(iv + 384)
            pe_fill = pe_a + pe_b
            act_sched = dict(act_a)
            for k, v in act_b.items():
                act_sched.setdefault(k + 16, []).extend(v)
            for u in range(32):
                emit_step(u, iv + 8 * u, pe_fill, act_sched.get(u))
            assert not pe_fill

        # ---------------- FC head + log_softmax ----------------
        hrelu = ptmp.tile([128, 256], BF16, tag="hrelu")
        nc.scalar.activation(hrelu, hT, AFT.Relu)

        nc.tensor.matmul(fc_ps, ones1[:, :BL], bfc_sb, start=True, stop=False)
        for kc in range(KT):
            nc.tensor.matmul(fc_ps, hrelu[:, 32 * kc : 32 * kc + BL],
                             wfc_sb[:, O * kc : O * (kc + 1)],
                             start=False, stop=(kc == KT - 1))

        mx = ptmp.tile([BL, 1], F32, tag="mx")
        nc.vector.tensor_reduce(mx, fc_ps, mybir.AxisListType.X,
                                mybir.AluOpType.max)
        tt = ptmp.tile([BL, O], F32, tag="tt")
        nc.vector.tensor_scalar(tt, fc_ps, mx, None, mybir.AluOpType.subtract)
        ex = ptmp.tile([BL, O], F32, tag="ex")
        nc.scalar.activation(ex, tt, AFT.Exp)
        sm = ptmp.tile([BL, 1], F32, tag="sm")
        nc.vector.tensor_reduce(sm, ex, mybir.AxisListType.X,
                                mybir.AluOpType.add)
        lsm = ptmp.tile([BL, 1], F32, tag="lsm")
        nc.scalar.activation(lsm, sm, AFT.Ln)
        res = ptmp.tile([BL, O], F32, tag="res")
        nc.vector.tensor_scalar(res, tt, lsm, None, mybir.AluOpType.subtract)
        nc.sync.dma_start(out=out_d[:, :], in_=res)

    nc.compile()
    return nc


def prep_inputs(x, h, Wz, bz, Wr, br, Wh, bh, Wfc, bfc, nsteps=S):
    """Host-side prep: shard + relayout. Returns per-core input maps."""
    f32, bf16 = np.float32, ml_dtypes.bfloat16
    x = np.asarray(x, f32)[:, :nsteps, :]
    h0 = np.asarray(h, f32)[:, 0, :]
    pcol = _pcol()
    pad_rows = 128 * PAD_CHUNKS

    gates_h = [np.asarray(Wr, f32)[I:], np.asarray(Wz, f32)[I:],
               np.asarray(Wh, f32)[I:]]
    gates_x = [np.asarray(Wr, f32)[:I], np.asarray(Wz, f32)[:I],
               np.asarray(Wh, f32)[:I]]
    gates_b = [np.asarray(br, f32), np.asarray(bz, f32), np.asarray(bh, f32)]

    whh_img = np.zeros((128, KT * G3), bf16)
    for kc in range(KT):
        for gt in range(3):
            whh_img[:, G3 * kc + 1024 * gt : G3 * kc + 1024 * (gt + 1)] = \
                gates_h[gt][128 * kc : 128 * (kc + 1), pcol]
    wx_imgf = np.zeros((128, KTI * G3), f32)
    for k in range(KTI):
        for gt in range(3):
            wx_imgf[:, G3 * k + 1024 * gt : G3 * k + 1024 * (gt + 1)] = \
                gates_x[gt][128 * k : 128 * (k + 1), pcol]
    wx_img = wx_imgf.astype(bf16)
    bias_img = np.concatenate([g[pcol] for g in gates_b])[None, :].astype(bf16)

    id8 = np.zeros((8, 32), bf16)
    np.fill_diagonal(id8[:, :8], 1)
    ones1 = np.ones((1, 128), bf16)
    wfc_img = np.asarray(Wfc, f32).reshape(KT, 128, O).transpose(1, 0, 2) \
        .reshape(128, KT * O).astype(bf16)
    bfc_img = np.asarray(bfc, f32)[None, :].astype(bf16)

    in_maps = []
    for c in range(NCORES):
        xc = x[c * BL : (c + 1) * BL]                      # [8, S', I]
        xtf = np.zeros((I, BL * nsteps + pad_rows), f32)
        xtf[:, : BL * nsteps] = xc.transpose(2, 1, 0).reshape(I, nsteps * BL)
        xt = xtf.astype(bf16)
        h0c = h0[c * BL : (c + 1) * BL]                    # [8, H]
        hv = h0c.reshape(BL, 8, 4, 32)                     # [b, m, g, i]
        h0sm = np.zeros((128, 256), bf16)
        h0st = np.zeros((128, 256), bf16)
        for g in range(4):
            h0sm[32 * g : 32 * g + BL, :] = hv[:, :, g, :].reshape(BL, 256)
            zt = np.zeros((32, 8, 32), f32)
            zt[:, :, :BL] = hv[:, :, g, :].transpose(2, 1, 0)
            h0st[32 * g : 32 * g + 32, :] = zt.reshape(32, 256)
        in_maps.append({
            "xt": xt, "xtf": xtf, "h0sm": h0sm, "h0st": h0st,
            "whh": whh_img, "wx": wx_img, "wxf": wx_imgf, "bias": bias_img,
            "id8": id8, "ones1": ones1,
            "wfc": wfc_img, "bfc": bfc_img,
        })
    return in_maps


_BUILT = {}
_LAST_RESULTS = None


def kernel(**inputs):
    global _LAST_RESULTS
    key = "full"
    if key not in _BUILT:
        _BUILT[key] = build()
    nc = _BUILT[key]
    in_maps = prep_inputs(**inputs)
    trace = bool(int(os.environ.get("BASS_TRACE", "0") or "0"))
    res = run_bass_kernel_spmd(nc, in_maps, list(range(NCORES)), trace=trace)
    _LAST_RESULTS = res
    outs = [res.results[c]["out"] for c in range(NCORES)]
    return np.concatenate(outs, axis=0).astype(np.float32)


if __name__ == "__main__":
    np.random.seed(0)
    print("building...")
    nc = build(2, num_devices=1)
    print("build ok:", nc)
